# revision 11
# baseline (speedup 1.0000x reference)
"""Trainium2 Bass kernel: causal multi-head attention with softmax over the
QUERY axis (faithful to the reference's softmax(dim=-2) quirk).

Problem shapes: B=2, T=2048, E=1024, H=16, D=64.

Sharding: 8 cores = 2 batches x 4 head-groups (4 heads per core, zero
communication).  Host pre-transposes X to (E, T) per batch, packs per-head
weights into head-pair columns, and reassembles the output from per-core
(2, 128, 2048) fp16 tiles.

V2 schedule (ACT-bound design):
  - Blocks processed in REVERSE (15..0) per pair: block 15 needs only the
    tails of Q and K, so the first exp lands a few us into the kernel
    instead of after the full QK projection (42us ACT idle in V1).
  - AV is DECOUPLED from exp: every e^S tile is kept in SBUF (fp16) and the
    O = V'E accumulation for pair 0 runs during pair 1's exp phase.  This
    frees PSUM banks for projection scratch while pair 0's exps run.
  - One ACTIVATE per (block, head): wide blocks (W>1024) write their scores
    into two 2-bank slots of a manually laid out 8-bank PSUM tensor and the
    exp reads both halves via one 3-level access pattern -> 64 activations
    instead of 96.
  - Causal mask added on PE (identity @ mask accumulating matmul), not DVE.
  - den (sum over queries) via ACT accum_out for narrow blocks and DVE
    tensor_reduce (2x fp16 rate) for wide blocks - balances the queues.

PSUM bank map (2KB banks, manual slices of one 8-bank tensor):
  phase 1 (pair-0 exps):  0:K-proj scratch  1:Q-proj scratch
                          2,3: V-proj scratch   4-5,6-7: score slots
                          (0-3 join the score-slot pool for wide blocks)
  phase 2 (pair-1 exps):  0-1: O0-high, then O0-low, then score slots
                          2-3: V/QK1 scratch, then O1-high
                          4-5,6-7: score slots
  tail:                   4-5: O1-low (replayed from saved e tiles)
"""

import numpy as np
from contextlib import ExitStack

B, T, E, H, D = 2, 2048, 1024, 16, 64
NCORES = 8
PAIRS = 2          # head pairs per core (4 heads)
EC = E // 128      # 8 contraction chunks
TB = T // 128      # 16 s-blocks
NEG = -60000.0     # mask add (fp16-safe); exp(SCALE*NEG) == 0
SCALE = float(D) ** -0.5

# den strategy: True -> DVE tensor_reduce on the fp16 e tile;
# False -> ACT accum_out (costs ~290ns READ_ACCUM on the scalar queue).
DVE_DEN_WIDE = True     # blocks 0..7  (W > 1024)
DVE_DEN_NARROW = False  # blocks 8..15 (W <= 1024)
MASK_ON_PE = True

_CACHE = {}


def _emit(tc, io):
    """Emit the kernel program into TileContext tc.  io: dict name -> AP."""
    import concourse.bass as bass
    import concourse.mybir as mybir

    nc = tc.nc
    fp32 = mybir.dt.float32
    fp16 = mybir.dt.float16
    AF = mybir.ActivationFunctionType
    ALU = mybir.AluOpType
    AX = mybir.AxisListType

    x_t, wq, wk, wv = io["x_t"], io["wq"], io["wk"], io["wv"]
    bq, bk, bv, out = io["bq"], io["bk"], io["bv"], io["out"]

    with ExitStack() as ctx:
        const = ctx.enter_context(tc.tile_pool(name="const", bufs=1))
        big = ctx.enter_context(tc.tile_pool(name="big", bufs=1))
        dpool = ctx.enter_context(tc.tile_pool(name="dpool", bufs=6))
        rpool = ctx.enter_context(tc.tile_pool(name="rpool", bufs=34))
        vpool = ctx.enter_context(tc.tile_pool(name="vpool", bufs=18))
        epool = ctx.enter_context(tc.tile_pool(name="epool", bufs=3))
        opool = ctx.enter_context(tc.tile_pool(name="opool", bufs=4))
        pall = ctx.enter_context(tc.tile_pool(name="pall", bufs=1,
                                              space="PSUM"))
        PS = pall.tile([128, 8, 512], fp32, tag="ps")

        def bankflat(b0, nb, w=None):
            """(128, nb*512) view of banks [b0, b0+nb)."""
            v = PS[:, b0:b0 + nb, :].rearrange("p a b -> p (a b)")
            return v if w is None else v[:, 0:w]

        # ---------- constants + inputs ----------
        wq_sb = const.tile([128, EC, 256], fp16, tag="wq")
        wk_sb = const.tile([128, EC, 256], fp16, tag="wk")
        wv_sb = const.tile([128, EC, 256], fp16, tag="wv")
        for w_sb_, w_ in ((wk_sb, wk), (wq_sb, wq), (wv_sb, wv)):
            nc.sync.dma_start(out=w_sb_,
                              in_=w_.rearrange("(c p) n -> p c n", p=128))

        bq_sb = const.tile([128, PAIRS], fp32, tag="bq")
        bk_sb = const.tile([128, PAIRS], fp32, tag="bk")
        for p in range(PAIRS):
            nc.sync.dma_start(out=bq_sb[:, p:p + 1], in_=bq[p, :, None])
            nc.sync.dma_start(out=bk_sb[:, p:p + 1], in_=bk[p, :, None])

        bv_sb = const.tile([128, 256], fp32, tag="bv")
        bv_bcast = bass.AP(tensor=bv.tensor, offset=bv.offset,
                           ap=[[0, 128]] + list(bv.ap))
        nc.gpsimd.dma_start(out=bv_sb, in_=bv_bcast)

        # X^T streamed by t-column blocks in REVERSE (tail first): the
        # reverse-order QK projections start as soon as the tail lands.
        xt_sb = big.tile([128, EC, T], fp16, tag="xt")
        xt_view = x_t.rearrange("(c p) t -> p c t", p=128)
        for j in (3, 2, 1, 0):
            nc.gpsimd.dma_start(out=xt_sb[:, :, 512 * j:512 * (j + 1)],
                                in_=xt_view[:, :, 512 * j:512 * (j + 1)])

        # zeros row used to open O accumulation banks
        zrow_sb = const.tile([1, 512], fp16, tag="zrow")
        nc.vector.memset(zrow_sb, 0.0)

        # PE warm-up during the input-DMA wait (HAM clock gate)
        warm_sb = const.tile([128, 512], fp16, tag="warm")
        nc.vector.memset(warm_sb, 0.0)
        for _ in range(24):
            nc.tensor.matmul(PS[:, 0, :], lhsT=warm_sb[:, 0:128],
                             rhs=warm_sb, start=True, stop=True)

        # ACT warm-up: load the exp table set outside the critical path
        wact_sb = const.tile([128, 2], fp16, tag="wact")
        nc.scalar.activation(out=wact_sb, in_=warm_sb[:, 0:2], func=AF.Exp)

        # mask[s, t] = NEG if t < s else 0; ident for the PE mask-add
        mask_sb = const.tile([128, 128], fp16, tag="mask")
        nc.vector.memset(mask_sb, 0.0)
        nc.gpsimd.affine_select(
            out=mask_sb, in_=mask_sb,
            pattern=[[1, 128]], channel_multiplier=-1, base=0,
            compare_op=ALU.is_ge, fill=NEG,
        )
        ident_sb = const.tile([128, 128], fp16, tag="ident")
        nc.vector.memset(ident_sb, 0.0)
        nc.gpsimd.affine_select(
            out=ident_sb, in_=ident_sb,
            pattern=[[1, 128]], channel_multiplier=-1, base=0,
            compare_op=ALU.not_equal, fill=1.0,
        )

        # ---------- persistent SBUF state ----------
        qt_sb = [big.tile([128, T], fp16, tag=f"qt{p}", name=f"qt{p}")
                 for p in range(PAIRS)]
        kt_sb = [big.tile([128, T], fp16, tag=f"kt{p}", name=f"kt{p}")
                 for p in range(PAIRS)]
        v_sb = big.tile([128, TB, 256], fp16, tag="v")

        esav = {}   # (p, i) -> e_tiles[2]
        rsav = {}   # (p, i) -> rec tile (128, 2) fp32  (1/den per head)
        vsav = {}   # (p, i) -> vp tile

        # ---------- emit helpers ----------
        def emit_qk_chunk(p, j, bank, which):
            """Project Q or K for pair p, t-cols [512j, 512j+512)."""
            w_sb, b_sb, dst = ((wq_sb, bq_sb, qt_sb[p]) if which == "q"
                               else (wk_sb, bk_sb, kt_sb[p]))
            ps = PS[:, bank, :]
            for c in range(EC):
                nc.tensor.matmul(
                    ps, lhsT=w_sb[:, c, 128 * p:128 * (p + 1)],
                    rhs=xt_sb[:, c, 512 * j:512 * (j + 1)],
                    start=(c == 0), stop=(c == EC - 1),
                )
            nc.vector.tensor_scalar_add(
                out=dst[:, 512 * j:512 * (j + 1)], in0=ps,
                scalar1=b_sb[:, p:p + 1])

        def emit_v_block(i, bank):
            """Project V block i (all 4 heads, 256 cols)."""
            ps = PS[:, bank, 0:256]
            for c in range(EC):
                nc.tensor.matmul(
                    ps, lhsT=xt_sb[:, c, 128 * i:128 * (i + 1)],
                    rhs=wv_sb[:, c, :],
                    start=(c == 0), stop=(c == EC - 1),
                )
            nc.vector.tensor_tensor(out=v_sb[:, i, :], in0=ps,
                                    in1=bv_sb, op=ALU.add)

        def emit_head_unit(p, i, h, pl, den_ap, e_tile):
            """Scores (2-quadrant MMs) + mask + one exp for head h of block
            i, pair p.  pl = [(bank0, abs_c0, w), ...] (1 or 2 pieces of
            equal width).  Adjacent units use different PE row groups, so
            their matmuls overlap on the array."""
            s0 = 128 * i
            W = (T - s0)
            for (b0, c0, cw) in pl:
                off = 0
                while off < cw:
                    w_ = min(512, cw - off)
                    for q in range(2):
                        nc.tensor.matmul(
                            bankflat(b0, 2)[64 * q:64 * (q + 1),
                                            off:off + w_],
                            lhsT=kt_sb[p][64 * h:64 * (h + 1),
                                          s0 + 64 * q:s0 + 64 * (q + 1)],
                            rhs=qt_sb[p][64 * h:64 * (h + 1),
                                         c0 + off:c0 + off + w_],
                            start=True, stop=True,
                            tile_position=(64 * h, 64 * q),
                            skip_group_check=True,
                        )
                    off += w_
            b0 = min(pl, key=lambda x: x[1])[0]
            if MASK_ON_PE:
                nc.tensor.matmul(
                    bankflat(b0, 2)[:, 0:128], lhsT=ident_sb, rhs=mask_sb,
                    start=False, stop=True, skip_group_check=True,
                )
            else:
                nc.vector.tensor_tensor(
                    out=bankflat(b0, 2)[:, 0:128],
                    in0=bankflat(b0, 2)[:, 0:128],
                    in1=mask_sb, op=ALU.add)
            if len(pl) == 1:
                in_ap = bankflat(pl[0][0], 2, W)
                out_ap = e_tile[:, 0:W]
            else:
                (bA, _, W2), (bB, _, _) = sorted(pl, key=lambda x: x[1])
                base = bankflat(bA, 2, W2)
                in_ap = bass.AP(
                    tensor=base.tensor, offset=base.offset,
                    ap=[list(base.ap[0]), [(bB - bA) * 512, 2],
                        list(base.ap[-1])])
                out_ap = e_tile[:, 0:W].rearrange("x (a b) -> x a b", a=2)
            nc.scalar.activation(
                out=out_ap, in_=in_ap, func=AF.Exp, scale=SCALE,
                accum_out=den_ap)

        def emit_block(p, i, slot_sets):
            """Scores + exps (+den/rec) for block i of pair p.
            slot_sets[h] = [bank0, ...] (1 slot narrow, 2 wide) per head."""
            s0 = 128 * i
            W = T - s0
            wide = W > 1024
            use_dve = DVE_DEN_WIDE if wide else DVE_DEN_NARROW
            den = dpool.tile([128, 2], fp32, tag="den",
                              name=f"den{p}_{i}")
            e_tiles = []
            for h in range(2):
                sl = slot_sets[h]
                if wide:
                    W2 = W // 2
                    blo, bhi = min(sl), max(sl)
                    colmap = {blo: s0, bhi: s0 + W2}
                    # write order follows sl (first entry = currently free
                    # bank pair); the exp AP needs ascending banks, so the
                    # column halves are assigned by bank order.
                    pl = [(sl[0], colmap[sl[0]], W2),
                          (sl[1], colmap[sl[1]], W2)]
                else:
                    pl = [(sl[0], s0, W)]
                e_tile = epool.tile([128, W], fp16, tag=f"e{W}",
                                    name=f"e{p}_{i}_{h}")
                e_tiles.append(e_tile)
                emit_head_unit(p, i, h, pl,
                               None if use_dve else den[:, h:h + 1], e_tile)
                if use_dve:
                    nc.vector.tensor_reduce(
                        out=den[:, h:h + 1], in_=e_tile[:, 0:W],
                        axis=AX.X, op=ALU.add)
            rec = rpool.tile([128, 2], fp32, tag="rec",
                             name=f"rec{p}_{i}")
            nc.vector.reciprocal(rec, den)
            esav[(p, i)] = e_tiles
            rsav[(p, i)] = rec

        def emit_vp(p, i):
            """Normalized V' block for (pair, block): (128 s, 128 d) fp16."""
            rec = rsav[(p, i)]
            vp = vpool.tile([128, 128], fp16, tag="vp",
                            name=f"vp{p}_{i}")
            for h in range(2):
                nc.vector.tensor_scalar_mul(
                    out=vp[:, 64 * h:64 * (h + 1)],
                    in0=v_sb[:, i, 128 * p + 64 * h:128 * p + 64 * (h + 1)],
                    scalar1=rec[:, h:h + 1])
            vsav[(p, i)] = vp

        def open_region(b0, nb):
            for b in range(b0, b0 + nb):
                nc.tensor.matmul(
                    PS[:, b, :], lhsT=zrow_sb[0:1, 0:128],
                    rhs=zrow_sb[0:1, 0:512],
                    start=True, stop=False, skip_group_check=True)

        def emit_av(p, i, b0, nb, abs0, c0, c1):
            """AV contribution of block i into O region at banks
            [b0, b0+nb) == absolute cols [abs0, abs0+nb*512); accumulate
            absolute cols [max(c0, s0), c1)."""
            s0 = 128 * i
            e_tiles = esav[(p, i)]
            vp = vsav[(p, i)]
            reg = bankflat(b0, nb)
            off = max(c0, s0)
            while off < c1:
                w_ = min(512 - (off % 512), c1 - off)
                for h in range(2):
                    nc.tensor.matmul(
                        reg[64 * h:64 * (h + 1),
                            off - abs0:off - abs0 + w_],
                        lhsT=vp[:, 64 * h:64 * (h + 1)],
                        rhs=e_tiles[h][:, off - s0:off - s0 + w_],
                        start=False, stop=False,
                        tile_position=(0, 64 * h),
                        skip_group_check=True,
                    )
                off += w_

        def flush_region(p, b0, nb, abs0):
            """PSUM O region -> SBUF (fp16) -> DRAM."""
            o_sb = opool.tile([128, nb * 512], fp16, tag="o",
                              name=f"o{p}_{abs0}")
            nc.vector.tensor_copy(o_sb, bankflat(b0, nb))
            nc.sync.dma_start(out=out[p][:, abs0:abs0 + nb * 512], in_=o_sb)

        # ---------- work queues (paced between blocks) ----------
        def make_drain(queue):
            state = {"n": 0}

            def drain(n):
                for _ in range(n):
                    if state["n"] >= len(queue):
                        return
                    queue[state["n"]]()
                    state["n"] += 1
            return drain

        # =========================================================
        # PHASE 1: pair-0 exps (blocks 15..0); projections interleaved.
        # =========================================================
        emit_qk_chunk(0, 3, 0, "k")
        emit_qk_chunk(0, 3, 1, "q")

        projq1 = [
            lambda: emit_qk_chunk(1, 3, 0, "k"),
            lambda: emit_qk_chunk(1, 3, 1, "q"),
            lambda: emit_qk_chunk(0, 2, 0, "k"),
            lambda: emit_qk_chunk(0, 2, 1, "q"),
            lambda: emit_v_block(15, 2), lambda: emit_v_block(14, 3),
            lambda: emit_qk_chunk(0, 1, 0, "k"),
            lambda: emit_qk_chunk(0, 1, 1, "q"),
            lambda: emit_v_block(13, 2), lambda: emit_v_block(12, 3),
            lambda: emit_qk_chunk(0, 0, 0, "k"),
            lambda: emit_qk_chunk(0, 0, 1, "q"),
            lambda: emit_v_block(11, 2), lambda: emit_v_block(10, 3),
            lambda: emit_v_block(9, 2), lambda: emit_v_block(8, 3),
        ]
        drain1 = make_drain(projq1)

        # narrow blocks: head-units rotate over slot-pairs 4-5 / 6-7
        for i in range(15, 7, -1):
            emit_block(0, i, [[4], [6]])
            drain1(2)
        drain1(len(projq1))              # banks 0-3 now free
        # wide blocks: head-units alternate {4,6} / {0,2} (unit depth 2:
        # head A is always on {4,6}, head B on {0,2})
        for i in range(7, -1, -1):
            emit_block(0, i, [[4, 6], [0, 2]])

        # =========================================================
        # PHASE 2: pair-1 exps; pair-0 AV + O0 accumulation; V tail.
        # =========================================================
        avq2 = []
        # O0-high: cols [1024, 2048), banks 0-1, all 16 blocks.
        avq2.append(lambda: open_region(0, 2))
        for i in list(range(15, 7, -1)) + list(range(7, -1, -1)):
            def f(i=i):
                emit_vp(0, i)
                emit_av(0, i, 0, 2, 1024, 1024, 2048)
            avq2.append(f)
        avq2.append(lambda: flush_region(0, 0, 2, 1024))
        # O0-low: cols [0, 1024), banks 0-1 reused, blocks 7..0 only.
        avq2.append(lambda: open_region(0, 2))
        for i in range(7, -1, -1):
            avq2.append(lambda i=i: emit_av(0, i, 0, 2, 0, 0, 1024))
        avq2.append(lambda: flush_region(0, 0, 2, 0))
        drain_av = make_drain(avq2)

        projq2 = [
            lambda: emit_qk_chunk(1, 2, 2, "k"),
            lambda: emit_qk_chunk(1, 2, 3, "q"),
            lambda: emit_v_block(7, 2), lambda: emit_v_block(6, 3),
            lambda: emit_qk_chunk(1, 1, 2, "k"),
            lambda: emit_qk_chunk(1, 1, 3, "q"),
            lambda: emit_v_block(5, 2), lambda: emit_v_block(4, 3),
            lambda: emit_qk_chunk(1, 0, 2, "k"),
            lambda: emit_qk_chunk(1, 0, 3, "q"),
            lambda: emit_v_block(3, 2), lambda: emit_v_block(2, 3),
            lambda: emit_v_block(1, 2), lambda: emit_v_block(0, 3),
        ]
        drain2 = make_drain(projq2)

        for i in range(15, 7, -1):       # narrow pair-1 blocks
            drain2(2)
            drain_av(2)
            emit_block(1, i, [[4], [6]])
        drain2(len(projq2))              # banks 2-3 free after this

        # wide pair-1 blocks: three slot-pairs {4-5, 6-7, 2-3} rotate at
        # piece granularity (unit k's high piece waits on exp k-1 ->
        # ~W2/2.4ns bubble per unit, unavoidable with 6 banks).  The O0
        # chain (avq2) finishes inside the first few iterations; O1-high
        # then opens on banks 0-1.
        rot = [(4, 6), (2, 4), (6, 2)]
        ui = 0
        o1h_open = {"done": False}

        def ensure_o1h():
            if not o1h_open["done"]:
                o1h_open["done"] = True
                drain_av(len(avq2))      # O0 fully done; banks 0-1 free
                open_region(0, 2)
                for k in range(15, 7, -1):   # replay narrow pair-1 blocks
                    emit_vp(1, k)
                    emit_av(1, k, 0, 2, 1024, 1024, 2048)

        pend_hi = []
        for i in range(7, -1, -1):
            ss = [list(rot[ui % 3]), list(rot[(ui + 1) % 3])]
            ui += 2
            if i >= 5:
                drain_av(6)
                emit_block(1, i, ss)
                pend_hi.append(i)
            else:
                ensure_o1h()
                emit_block(1, i, ss)
                while pend_hi:
                    k = pend_hi.pop(0)
                    emit_vp(1, k)
                    emit_av(1, k, 0, 2, 1024, 1024, 2048)
                emit_vp(1, i)
                emit_av(1, i, 0, 2, 1024, 1024, 2048)
        flush_region(1, 0, 2, 1024)

        # =========================================================
        # TAIL: O1-low from saved pair-1 wide e tiles (banks 4-5 free).
        # =========================================================
        open_region(4, 2)
        for i in range(7, -1, -1):
            emit_av(1, i, 4, 2, 0, 0, 1024)
        flush_region(1, 4, 2, 0)


def _build():
    """Build + schedule + compile the (SPMD-identical) program once."""
    if "nc" in _CACHE:
        return _CACHE["nc"]
    import concourse.bacc as bacc
    import concourse.mybir as mybir
    import concourse.tile as tile

    fp32 = mybir.dt.float32
    fp16 = mybir.dt.float16
    nc = bacc.Bacc("TRN2", target_bir_lowering=False, debug=False)
    io = {
        "x_t": nc.dram_tensor("x_t", [E, T], fp16, kind="ExternalInput").ap(),
        "wq": nc.dram_tensor("wq", [E, 256], fp16, kind="ExternalInput").ap(),
        "wk": nc.dram_tensor("wk", [E, 256], fp16, kind="ExternalInput").ap(),
        "wv": nc.dram_tensor("wv", [E, 256], fp16, kind="ExternalInput").ap(),
        "bq": nc.dram_tensor("bq", [PAIRS, 128], fp32,
                             kind="ExternalInput").ap(),
        "bk": nc.dram_tensor("bk", [PAIRS, 128], fp32,
                             kind="ExternalInput").ap(),
        "bv": nc.dram_tensor("bv", [256], fp32, kind="ExternalInput").ap(),
        "out": nc.dram_tensor("out", [PAIRS, 128, T], fp16,
                              kind="ExternalOutput").ap(),
    }
    with tile.TileContext(nc) as tc:
        _emit(tc, io)
    nc.compile()
    _CACHE["nc"] = nc
    return nc


def make_in_maps(inputs_x, k_w, k_b, q_w, q_b, v_w, v_b):
    """Host-side sharding: per-core input dict."""
    xs = [np.ascontiguousarray(inputs_x[b].T.astype(np.float16))
          for b in range(B)]
    in_maps = []
    for core in range(NCORES):
        b, g = divmod(core, 4)
        hs = range(4 * g, 4 * g + 4)
        pack_w = lambda w: np.ascontiguousarray(
            np.concatenate([w[h] for h in hs], axis=1).astype(np.float16))
        pack_b2 = lambda bb: np.ascontiguousarray(
            bb[4 * g:4 * g + 4].reshape(PAIRS, 128).astype(np.float32,
                                                           copy=False))
        in_maps.append({
            "x_t": xs[b],
            "wq": pack_w(q_w), "wk": pack_w(k_w), "wv": pack_w(v_w),
            "bq": pack_b2(q_b), "bk": pack_b2(k_b),
            "bv": np.ascontiguousarray(
                v_b[4 * g:4 * g + 4].reshape(256).astype(np.float32,
                                                         copy=False)),
        })
    return in_maps


def assemble(core_outs):
    """Gather per-core (PAIRS, 128, T) outputs into the full (B, T, H*D)."""
    out_full = np.empty((B, T, H * D), np.float32)
    for core in range(NCORES):
        b, g = divmod(core, 4)
        o = core_outs[core]
        for p in range(PAIRS):
            out_full[b, :, g * 256 + 128 * p:g * 256 + 128 * (p + 1)] = \
                o[p].T.astype(np.float32)
    return out_full


def kernel(**inputs):
    x = np.asarray(inputs["inputs"], np.float32)
    args = [np.asarray(inputs[k], np.float32)
            for k in ("k_w", "k_b", "q_w", "q_b", "v_w", "v_b")]
    from concourse.bass_utils import run_bass_kernel_spmd

    nc = _build()
    in_maps = make_in_maps(x, *args)
    res = run_bass_kernel_spmd(nc, in_maps, core_ids=list(range(NCORES)))
    return assemble([r["out"] for r in res.results])


# revision 12
# speedup vs baseline: 1.0470x; 1.0470x over previous
"""Trainium2 Bass kernel: causal multi-head attention with softmax over the
QUERY axis (faithful to the reference's softmax(dim=-2) quirk).

Problem shapes: B=2, T=2048, E=1024, H=16, D=64.

Sharding: 8 cores = 2 batches x 4 head-groups (4 heads per core, zero
communication).  Host pre-transposes X to (E, T) per batch, packs per-head
weights into head-pair columns, and reassembles the output from per-core
(2, 128, 2048) fp16 tiles.

V2 schedule (ACT-bound design):
  - Blocks processed in REVERSE (15..0) per pair: block 15 needs only the
    tails of Q and K, so the first exp lands a few us into the kernel
    instead of after the full QK projection (42us ACT idle in V1).
  - AV is DECOUPLED from exp: every e^S tile is kept in SBUF (fp16) and the
    O = V'E accumulation for pair 0 runs during pair 1's exp phase.  This
    frees PSUM banks for projection scratch while pair 0's exps run.
  - One ACTIVATE per (block, head): wide blocks (W>1024) write their scores
    into two 2-bank slots of a manually laid out 8-bank PSUM tensor and the
    exp reads both halves via one 3-level access pattern -> 64 activations
    instead of 96.
  - Causal mask added on PE (identity @ mask accumulating matmul), not DVE.
  - den (sum over queries) via ACT accum_out for narrow blocks and DVE
    tensor_reduce (2x fp16 rate) for wide blocks - balances the queues.

PSUM bank map (2KB banks, manual slices of one 8-bank tensor):
  phase 1 (pair-0 exps):  0:K-proj scratch  1:Q-proj scratch
                          2,3: V-proj scratch   4-5,6-7: score slots
                          (0-3 join the score-slot pool for wide blocks)
  phase 2 (pair-1 exps):  0-1: O0-high, then O0-low, then score slots
                          2-3: V/QK1 scratch, then O1-high
                          4-5,6-7: score slots
  tail:                   4-5: O1-low (replayed from saved e tiles)
"""

import numpy as np
from contextlib import ExitStack

B, T, E, H, D = 2, 2048, 1024, 16, 64
NCORES = 8
PAIRS = 2          # head pairs per core (4 heads)
EC = E // 128      # 8 contraction chunks
TB = T // 128      # 16 s-blocks
NEG = -60000.0     # mask add (fp16-safe); exp(SCALE*NEG) == 0
SCALE = float(D) ** -0.5

# den strategy: True -> DVE tensor_reduce on the fp16 e tile;
# False -> ACT accum_out (costs ~290ns READ_ACCUM on the scalar queue).
DVE_DEN_WIDE = False    # blocks 0..7  (W > 1024)
DVE_DEN_NARROW = False  # blocks 8..15 (W <= 1024)
MASK_ON_PE = True

_CACHE = {}


def _emit(tc, io):
    """Emit the kernel program into TileContext tc.  io: dict name -> AP."""
    import concourse.bass as bass
    import concourse.mybir as mybir

    nc = tc.nc
    fp32 = mybir.dt.float32
    fp16 = mybir.dt.float16
    AF = mybir.ActivationFunctionType
    ALU = mybir.AluOpType
    AX = mybir.AxisListType

    x_t, wq, wk, wv = io["x_t"], io["wq"], io["wk"], io["wv"]
    bv, out = io["bv"], io["out"]

    with ExitStack() as ctx:
        const = ctx.enter_context(tc.tile_pool(name="const", bufs=1))
        big = ctx.enter_context(tc.tile_pool(name="big", bufs=1))
        dpool = ctx.enter_context(tc.tile_pool(name="dpool", bufs=6))
        rpool = ctx.enter_context(tc.tile_pool(name="rpool", bufs=34))
        vpool = ctx.enter_context(tc.tile_pool(name="vpool", bufs=18))
        epool = ctx.enter_context(tc.tile_pool(name="epool", bufs=3))
        opool = ctx.enter_context(tc.tile_pool(name="opool", bufs=4))
        pall = ctx.enter_context(tc.tile_pool(name="pall", bufs=1,
                                              space="PSUM"))
        PS = pall.tile([128, 8, 512], fp32, tag="ps")

        def bankflat(b0, nb, w=None):
            """(128, nb*512) view of banks [b0, b0+nb)."""
            v = PS[:, b0:b0 + nb, :].rearrange("p a b -> p (a b)")
            return v if w is None else v[:, 0:w]

        # ---------- constants + inputs ----------
        wq_sb = const.tile([128, EC, 256], fp16, tag="wq")
        wk_sb = const.tile([128, EC, 256], fp16, tag="wk")
        wv_sb = const.tile([128, EC, 256], fp16, tag="wv")
        # bias first (tiny, unblocks the DVE bias-adds immediately)
        bqk_sb = const.tile([128, 4], fp32, tag="bqk")
        nc.sync.dma_start(out=bqk_sb, in_=io["bqk"])
        bq_sb, bk_sb = bqk_sb[:, 0:2], bqk_sb[:, 2:4]

        bv_sb = const.tile([128, 256], fp32, tag="bv")
        bv_bcast = bass.AP(tensor=bv.tensor, offset=bv.offset,
                           ap=[[0, 128]] + list(bv.ap))
        nc.gpsimd.dma_start(out=bv_sb, in_=bv_bcast)

        # X^T streamed by t-column blocks in REVERSE (tail first).  Host
        # supplies [j, p, c, t] so each chunk is 8KB-contiguous per
        # partition; SBUF layout is [p, j, c, t] for the same reason.
        xt_sb = big.tile([128, 4, EC, 512], fp16, tag="xt")
        nc.sync.dma_start(out=wk_sb, in_=wk)
        nc.sync.dma_start(out=wq_sb, in_=wq)
        nc.gpsimd.dma_start(out=xt_sb[:, 3], in_=x_t[3])
        nc.sync.dma_start(out=wv_sb, in_=wv)
        nc.gpsimd.dma_start(out=xt_sb[:, 2], in_=x_t[2])
        nc.sync.dma_start(out=xt_sb[:, 1], in_=x_t[1])
        nc.gpsimd.dma_start(out=xt_sb[:, 0], in_=x_t[0])

        # zeros row used to open O accumulation banks
        zrow_sb = const.tile([1, 512], fp16, tag="zrow")
        nc.vector.memset(zrow_sb, 0.0)

        # PE warm-up during the input-DMA wait (HAM clock gate)
        warm_sb = const.tile([128, 512], fp16, tag="warm")
        nc.vector.memset(warm_sb, 0.0)
        for _ in range(10):
            nc.tensor.matmul(PS[:, 0, :], lhsT=warm_sb[:, 0:128],
                             rhs=warm_sb, start=True, stop=True)

        # ACT warm-up: load the exp table set outside the critical path
        wact_sb = const.tile([128, 2], fp16, tag="wact")
        nc.scalar.activation(out=wact_sb, in_=warm_sb[:, 0:2], func=AF.Exp)

        # mask[s, t] = NEG if t < s else 0; ident for the PE mask-add
        mask_sb = const.tile([128, 128], fp16, tag="mask")
        nc.vector.memset(mask_sb, 0.0)
        nc.gpsimd.affine_select(
            out=mask_sb, in_=mask_sb,
            pattern=[[1, 128]], channel_multiplier=-1, base=0,
            compare_op=ALU.is_ge, fill=NEG,
        )
        ident_sb = const.tile([128, 128], fp16, tag="ident")
        nc.vector.memset(ident_sb, 0.0)
        nc.gpsimd.affine_select(
            out=ident_sb, in_=ident_sb,
            pattern=[[1, 128]], channel_multiplier=-1, base=0,
            compare_op=ALU.not_equal, fill=1.0,
        )

        # ---------- persistent SBUF state ----------
        qt_sb = [big.tile([128, T], fp16, tag=f"qt{p}", name=f"qt{p}")
                 for p in range(PAIRS)]
        kt_sb = [big.tile([128, T], fp16, tag=f"kt{p}", name=f"kt{p}")
                 for p in range(PAIRS)]
        v_sb = big.tile([128, TB, 256], fp16, tag="v")

        esav = {}   # (p, i) -> e_tiles[2]
        rsav = {}   # (p, i) -> rec tile (128, 2) fp32  (1/den per head)
        vsav = {}   # (p, i) -> vp tile

        # ---------- emit helpers ----------
        def emit_qk_chunk(p, j, bank, which):
            """Project Q or K for pair p, t-cols [512j, 512j+512)."""
            w_sb, b_sb, dst = ((wq_sb, bq_sb, qt_sb[p]) if which == "q"
                               else (wk_sb, bk_sb, kt_sb[p]))
            ps = PS[:, bank, :]
            for c in range(EC):
                nc.tensor.matmul(
                    ps, lhsT=w_sb[:, c, 128 * p:128 * (p + 1)],
                    rhs=xt_sb[:, j, c, :],
                    start=(c == 0), stop=(c == EC - 1),
                )
            nc.vector.tensor_scalar_add(
                out=dst[:, 512 * j:512 * (j + 1)], in0=ps,
                scalar1=b_sb[:, p:p + 1])

        def emit_v_block(i, bank):
            """Project V block i (all 4 heads, 256 cols)."""
            ps = PS[:, bank, 0:256]
            for c in range(EC):
                nc.tensor.matmul(
                    ps, lhsT=xt_sb[:, i // 4, c, 128 * (i % 4):
                                   128 * (i % 4) + 128],
                    rhs=wv_sb[:, c, :],
                    start=(c == 0), stop=(c == EC - 1),
                )
            nc.vector.tensor_tensor(out=v_sb[:, i, :], in0=ps,
                                    in1=bv_sb, op=ALU.add)

        def emit_head_unit(p, i, h, pl, den_ap, e_tile):
            """Scores (2-quadrant MMs) + mask + one exp for head h of block
            i, pair p.  pl = [(bank0, abs_c0, w), ...] (1 or 2 pieces of
            equal width).  Adjacent units use different PE row groups, so
            their matmuls overlap on the array."""
            s0 = 128 * i
            W = (T - s0)
            for (b0, c0, cw) in pl:
                off = 0
                while off < cw:
                    w_ = min(512, cw - off)
                    for q in range(2):
                        nc.tensor.matmul(
                            bankflat(b0, 2)[64 * q:64 * (q + 1),
                                            off:off + w_],
                            lhsT=kt_sb[p][64 * h:64 * (h + 1),
                                          s0 + 64 * q:s0 + 64 * (q + 1)],
                            rhs=qt_sb[p][64 * h:64 * (h + 1),
                                         c0 + off:c0 + off + w_],
                            start=True, stop=True,
                            tile_position=(64 * h, 64 * q),
                            skip_group_check=True,
                        )
                    off += w_
            b0 = min(pl, key=lambda x: x[1])[0]
            if MASK_ON_PE:
                nc.tensor.matmul(
                    bankflat(b0, 2)[:, 0:128], lhsT=ident_sb, rhs=mask_sb,
                    start=False, stop=True, skip_group_check=True,
                )
            else:
                nc.vector.tensor_tensor(
                    out=bankflat(b0, 2)[:, 0:128],
                    in0=bankflat(b0, 2)[:, 0:128],
                    in1=mask_sb, op=ALU.add)
            if len(pl) == 1:
                in_ap = bankflat(pl[0][0], 2, W)
                out_ap = e_tile[:, 0:W]
            else:
                (bA, _, W2), (bB, _, _) = sorted(pl, key=lambda x: x[1])
                base = bankflat(bA, 2, W2)
                in_ap = bass.AP(
                    tensor=base.tensor, offset=base.offset,
                    ap=[list(base.ap[0]), [(bB - bA) * 512, 2],
                        list(base.ap[-1])])
                out_ap = e_tile[:, 0:W].rearrange("x (a b) -> x a b", a=2)
            nc.scalar.activation(
                out=out_ap, in_=in_ap, func=AF.Exp, scale=SCALE,
                accum_out=den_ap)

        def emit_block(p, i, slot_sets):
            """Scores + exps (+den/rec) for block i of pair p.
            slot_sets[h] = [bank0, ...] (1 slot narrow, 2 wide) per head."""
            s0 = 128 * i
            W = T - s0
            wide = W > 1024
            use_dve = DVE_DEN_WIDE if wide else DVE_DEN_NARROW
            den = dpool.tile([128, 2], fp32, tag="den",
                              name=f"den{p}_{i}")
            e_tiles = []
            for h in range(2):
                sl = slot_sets[h]
                if wide:
                    W2 = W // 2
                    blo, bhi = min(sl), max(sl)
                    colmap = {blo: s0, bhi: s0 + W2}
                    # write order follows sl (first entry = currently free
                    # bank pair); the exp AP needs ascending banks, so the
                    # column halves are assigned by bank order.
                    pl = [(sl[0], colmap[sl[0]], W2),
                          (sl[1], colmap[sl[1]], W2)]
                else:
                    pl = [(sl[0], s0, W)]
                e_tile = epool.tile([128, W], fp16, tag=f"e{W}",
                                    name=f"e{p}_{i}_{h}")
                e_tiles.append(e_tile)
                emit_head_unit(p, i, h, pl,
                               None if use_dve else den[:, h:h + 1], e_tile)
                if use_dve:
                    nc.vector.tensor_reduce(
                        out=den[:, h:h + 1], in_=e_tile[:, 0:W],
                        axis=AX.X, op=ALU.add)
            rec = rpool.tile([128, 2], fp32, tag="rec",
                             name=f"rec{p}_{i}")
            nc.vector.reciprocal(rec, den)
            esav[(p, i)] = e_tiles
            rsav[(p, i)] = rec

        def emit_vp(p, i):
            """Normalized V' block for (pair, block): (128 s, 128 d) fp16."""
            rec = rsav[(p, i)]
            vp = vpool.tile([128, 128], fp16, tag="vp",
                            name=f"vp{p}_{i}")
            for h in range(2):
                nc.vector.tensor_scalar_mul(
                    out=vp[:, 64 * h:64 * (h + 1)],
                    in0=v_sb[:, i, 128 * p + 64 * h:128 * p + 64 * (h + 1)],
                    scalar1=rec[:, h:h + 1])
            vsav[(p, i)] = vp

        def open_region(b0, nb):
            for b in range(b0, b0 + nb):
                nc.tensor.matmul(
                    PS[:, b, :], lhsT=zrow_sb[0:1, 0:128],
                    rhs=zrow_sb[0:1, 0:512],
                    start=True, stop=False, skip_group_check=True)

        def emit_av(p, i, b0, nb, abs0, c0, c1):
            """AV contribution of block i into O region at banks
            [b0, b0+nb) == absolute cols [abs0, abs0+nb*512); accumulate
            absolute cols [max(c0, s0), c1)."""
            s0 = 128 * i
            e_tiles = esav[(p, i)]
            vp = vsav[(p, i)]
            reg = bankflat(b0, nb)
            off = max(c0, s0)
            while off < c1:
                w_ = min(512 - (off % 512), c1 - off)
                for h in range(2):
                    nc.tensor.matmul(
                        reg[64 * h:64 * (h + 1),
                            off - abs0:off - abs0 + w_],
                        lhsT=vp[:, 64 * h:64 * (h + 1)],
                        rhs=e_tiles[h][:, off - s0:off - s0 + w_],
                        start=False, stop=False,
                        tile_position=(0, 64 * h),
                        skip_group_check=True,
                    )
                off += w_

        def flush_region(p, b0, nb, abs0):
            """PSUM O region -> SBUF (fp16) -> DRAM."""
            o_sb = opool.tile([128, nb * 512], fp16, tag="o",
                              name=f"o{p}_{abs0}")
            nc.vector.tensor_copy(o_sb, bankflat(b0, nb))
            nc.sync.dma_start(out=out[p][:, abs0:abs0 + nb * 512], in_=o_sb)

        # ---------- work queues (paced between blocks) ----------
        def make_drain(queue):
            state = {"n": 0}

            def drain(n):
                for _ in range(n):
                    if state["n"] >= len(queue):
                        return
                    queue[state["n"]]()
                    state["n"] += 1
            return drain

        # =========================================================
        # PHASE 1: pair-0 exps (blocks 15..0); projections interleaved.
        # =========================================================
        emit_qk_chunk(0, 3, 0, "k")
        emit_qk_chunk(0, 3, 1, "q")

        projq1 = [
            lambda: emit_qk_chunk(1, 3, 0, "k"),
            lambda: emit_qk_chunk(1, 3, 1, "q"),
            lambda: emit_qk_chunk(0, 2, 0, "k"),
            lambda: emit_qk_chunk(0, 2, 1, "q"),
            lambda: emit_v_block(15, 2), lambda: emit_v_block(14, 3),
            lambda: emit_qk_chunk(0, 1, 0, "k"),
            lambda: emit_qk_chunk(0, 1, 1, "q"),
            lambda: emit_v_block(13, 2), lambda: emit_v_block(12, 3),
            lambda: emit_qk_chunk(0, 0, 0, "k"),
            lambda: emit_qk_chunk(0, 0, 1, "q"),
            lambda: emit_v_block(11, 2), lambda: emit_v_block(10, 3),
            lambda: emit_v_block(9, 2), lambda: emit_v_block(8, 3),
        ]
        drain1 = make_drain(projq1)

        # narrow blocks: head-units rotate over slot-pairs 4-5 / 6-7
        for i in range(15, 7, -1):
            emit_block(0, i, [[4], [6]])
            drain1(2)
        drain1(len(projq1))              # banks 0-3 now free
        # wide blocks: head-units alternate {4,6} / {0,2} (unit depth 2:
        # head A is always on {4,6}, head B on {0,2})
        for i in range(7, -1, -1):
            emit_block(0, i, [[4, 6], [0, 2]])

        # =========================================================
        # PHASE 2: pair-1 exps; pair-0 AV + O0 accumulation; V tail.
        # =========================================================
        avq2 = []
        # O0-high: cols [1024, 2048), banks 0-1, all 16 blocks.
        avq2.append(lambda: open_region(0, 2))
        for i in list(range(15, 7, -1)) + list(range(7, -1, -1)):
            def f(i=i):
                emit_vp(0, i)
                emit_av(0, i, 0, 2, 1024, 1024, 2048)
            avq2.append(f)
        avq2.append(lambda: flush_region(0, 0, 2, 1024))
        # O0-low: cols [0, 1024), banks 0-1 reused, blocks 7..0 only.
        avq2.append(lambda: open_region(0, 2))
        for i in range(7, -1, -1):
            avq2.append(lambda i=i: emit_av(0, i, 0, 2, 0, 0, 1024))
        avq2.append(lambda: flush_region(0, 0, 2, 0))
        drain_av = make_drain(avq2)

        projq2 = [
            lambda: emit_qk_chunk(1, 2, 2, "k"),
            lambda: emit_qk_chunk(1, 2, 3, "q"),
            lambda: emit_v_block(7, 2), lambda: emit_v_block(6, 3),
            lambda: emit_qk_chunk(1, 1, 2, "k"),
            lambda: emit_qk_chunk(1, 1, 3, "q"),
            lambda: emit_v_block(5, 2), lambda: emit_v_block(4, 3),
            lambda: emit_qk_chunk(1, 0, 2, "k"),
            lambda: emit_qk_chunk(1, 0, 3, "q"),
            lambda: emit_v_block(3, 2), lambda: emit_v_block(2, 3),
            lambda: emit_v_block(1, 2), lambda: emit_v_block(0, 3),
        ]
        drain2 = make_drain(projq2)

        for i in range(15, 7, -1):       # narrow pair-1 blocks
            drain2(2)
            drain_av(2)
            emit_block(1, i, [[4], [6]])
        drain2(len(projq2))              # banks 2-3 free after this

        # wide pair-1 blocks: three slot-pairs {4-5, 6-7, 2-3} rotate at
        # piece granularity (unit k's high piece waits on exp k-1 ->
        # ~W2/2.4ns bubble per unit, unavoidable with 6 banks).  The O0
        # chain (avq2) finishes inside the first few iterations; O1-high
        # then opens on banks 0-1.
        rot = [(4, 6), (2, 4), (6, 2)]
        ui = 0
        o1h_open = {"done": False}

        def ensure_o1h():
            if not o1h_open["done"]:
                o1h_open["done"] = True
                drain_av(len(avq2))      # O0 fully done; banks 0-1 free
                open_region(0, 2)
                for k in range(15, 7, -1):   # replay narrow pair-1 blocks
                    emit_vp(1, k)
                    emit_av(1, k, 0, 2, 1024, 1024, 2048)

        pend_hi = []
        for i in range(7, -1, -1):
            ss = [list(rot[ui % 3]), list(rot[(ui + 1) % 3])]
            ui += 2
            if i >= 5:
                drain_av(6)
                emit_block(1, i, ss)
                pend_hi.append(i)
            else:
                ensure_o1h()
                emit_block(1, i, ss)
                while pend_hi:
                    k = pend_hi.pop(0)
                    emit_vp(1, k)
                    emit_av(1, k, 0, 2, 1024, 1024, 2048)
                emit_vp(1, i)
                emit_av(1, i, 0, 2, 1024, 1024, 2048)
        flush_region(1, 0, 2, 1024)

        # =========================================================
        # TAIL: O1-low from saved pair-1 wide e tiles (banks 4-5 free).
        # =========================================================
        open_region(4, 2)
        for i in range(7, -1, -1):
            emit_av(1, i, 4, 2, 0, 0, 1024)
        flush_region(1, 4, 2, 0)


def _build():
    """Build + schedule + compile the (SPMD-identical) program once."""
    if "nc" in _CACHE:
        return _CACHE["nc"]
    import concourse.bacc as bacc
    import concourse.mybir as mybir
    import concourse.tile as tile

    fp32 = mybir.dt.float32
    fp16 = mybir.dt.float16
    nc = bacc.Bacc("TRN2", target_bir_lowering=False, debug=False)
    io = {
        "x_t": nc.dram_tensor("x_t", [4, 128, EC, 512], fp16,
                              kind="ExternalInput").ap(),
        "wq": nc.dram_tensor("wq", [128, EC, 256], fp16,
                             kind="ExternalInput").ap(),
        "wk": nc.dram_tensor("wk", [128, EC, 256], fp16,
                             kind="ExternalInput").ap(),
        "wv": nc.dram_tensor("wv", [128, EC, 256], fp16,
                             kind="ExternalInput").ap(),
        "bqk": nc.dram_tensor("bqk", [128, 4], fp32,
                              kind="ExternalInput").ap(),
        "bv": nc.dram_tensor("bv", [256], fp32, kind="ExternalInput").ap(),
        "out": nc.dram_tensor("out", [PAIRS, 128, T], fp16,
                              kind="ExternalOutput").ap(),
    }
    with tile.TileContext(nc) as tc:
        _emit(tc, io)
    nc.compile()
    _CACHE["nc"] = nc
    return nc


def make_in_maps(inputs_x, k_w, k_b, q_w, q_b, v_w, v_b):
    """Host-side sharding: per-core input dict."""
    xs = [np.ascontiguousarray(
              inputs_x[b].T.astype(np.float16)          # (E, T)
              .reshape(EC, 128, 4, 512).transpose(2, 1, 0, 3))
          for b in range(B)]
    in_maps = []
    for core in range(NCORES):
        b, g = divmod(core, 4)
        hs = range(4 * g, 4 * g + 4)
        pack_w = lambda w: np.ascontiguousarray(
            np.concatenate([w[h] for h in hs], axis=1).astype(np.float16)
            .reshape(EC, 128, 256).transpose(1, 0, 2))
        pack_b2 = lambda bb: bb[4 * g:4 * g + 4].reshape(PAIRS, 128).T
        in_maps.append({
            "x_t": xs[b],
            "wq": pack_w(q_w), "wk": pack_w(k_w), "wv": pack_w(v_w),
            "bqk": np.ascontiguousarray(
                np.concatenate([pack_b2(q_b), pack_b2(k_b)], axis=1)
                .astype(np.float32)),
            "bv": np.ascontiguousarray(
                v_b[4 * g:4 * g + 4].reshape(256).astype(np.float32,
                                                         copy=False)),
        })
    return in_maps


def assemble(core_outs):
    """Gather per-core (PAIRS, 128, T) outputs into the full (B, T, H*D)."""
    out_full = np.empty((B, T, H * D), np.float32)
    for core in range(NCORES):
        b, g = divmod(core, 4)
        o = core_outs[core]
        for p in range(PAIRS):
            out_full[b, :, g * 256 + 128 * p:g * 256 + 128 * (p + 1)] = \
                o[p].T.astype(np.float32)
    return out_full


def kernel(**inputs):
    x = np.asarray(inputs["inputs"], np.float32)
    args = [np.asarray(inputs[k], np.float32)
            for k in ("k_w", "k_b", "q_w", "q_b", "v_w", "v_b")]
    from concourse.bass_utils import run_bass_kernel_spmd

    nc = _build()
    in_maps = make_in_maps(x, *args)
    res = run_bass_kernel_spmd(nc, in_maps, core_ids=list(range(NCORES)))
    return assemble([r["out"] for r in res.results])
